# revision 39
# baseline (speedup 1.0000x reference)
"""Gated axial attention (height) Trainium2 kernel.

N,C,H,W = 16,128,128,128. 8 NeuronCores, data-parallel over batch N
(2 batches per core). All math per (core, batch n):

  q~ = (Wq/d) @ x          [c,(i,j)]   (d = sqrt(C))
  k  =  Wk    @ x          [c,(h,j)]
  vT_j[h,c] = sum_c' Gv1*Wv[c,c'] x[c',h,j]      (per-j matmul, transposed v)
  Eq = exp(q~_j^T k_j)     stored [h,(i,j)] via strided-dest ACT
  Sr_i = (Gq*rq_i)^T q~_i + (Gk/d*rk_i)^T k_i    (per-i matmul, PSUM accum)
  E  = Eq * exp(Sr)        (DVE mul, in-place into Eq)
  sig[h,i] = sum_j E ; R = 1/sig ; Wn = E * R[h,i]
  out_j[c,i] += vT_j^T Wn_j   (per-j matmul -> strided add)
  out_i[c,j] += rv_i^T Wn_i   (per-i matmul -> contiguous copy)

Host<->device transport is the bottleneck (axon tunnel ~50MB/s, shared
both directions), so the dispatch path minimizes wire bytes and RPCs:
  - x is quantized host-side to int8 with per-(n,c) power-of-two scales
    embedded in the payload as 2 trailing exponent columns (34MB instead
    of 134MB f32), uploaded per-core-chunk overlapped with quantization,
    and dequantized to bf16 inside the bass kernel (ACT copy with
    per-partition scale decoded via Exp).
  - y is quantized inside the bass kernel to int8 with per-(n,c,i) row
    scales stored as bf16 bytes in the same int8 output tensor; one
    34.1MB download, decoded host-side with threads.
  - replicated params (1x1 conv weights, rq/rk/rv) are uploaded once to
    core 0 and broadcast device-to-device; the device copies are cached
    across calls and revalidated with exact np.array_equal.
  - all jax jits are built once and cached in module state (the stock
    run_bass_kernel_spmd builds a fresh jit closure per call), the
    donated output buffer is recycled from the previous call, and steady
    state is a single exec dispatch per call.
End-to-end int8 quantization error is ~9.3e-3 (gate: 2e-2).
"""

import threading
from concurrent.futures import ThreadPoolExecutor

import numpy as np
import ml_dtypes

import jax
import jax.numpy as jnp
from jax.sharding import Mesh, PartitionSpec as P, NamedSharding

import concourse.bass as bass
import concourse.tile as tile
from concourse import bacc, mybir, bass2jax

N, C, H, W = 16, 128, 128, 128
HW = H * W
N_CORES = 8
NPC = N // N_CORES  # batches per core
BF16 = mybir.dt.bfloat16
F32 = mybir.dt.float32
ICHUNK = 32  # i-block streamed for rq/rk/rv

_PROG = None
_STATE = None
_LOCK = threading.Lock()


I8 = mybir.dt.int8
LN2_8 = float(np.log(2.0) / 8.0)
SCALE_Q = 126.99


def _build():
    nc = bacc.Bacc("TRN2", target_bir_lowering=False, debug=False,
                   num_devices=N_CORES)
    # packed int8 x: per (n,c) row = 16384 int8 values | c0 | c1, where the
    # dequant scale is 2**((c0*127+c1)/8)
    x_ap = nc.dram_tensor("x2", [NPC, C, HW + 2], I8,
                          kind="ExternalInput").ap()
    wq_ap = nc.dram_tensor("wqt", [C, C], BF16, kind="ExternalInput").ap()
    wk_ap = nc.dram_tensor("wkt", [C, C], BF16, kind="ExternalInput").ap()
    wv_ap = nc.dram_tensor("wvt", [C, C], BF16, kind="ExternalInput").ap()
    rq_ap = nc.dram_tensor("rqh", [C, HW], BF16, kind="ExternalInput").ap()
    rk_ap = nc.dram_tensor("rkh", [C, HW], BF16, kind="ExternalInput").ap()
    rv_ap = nc.dram_tensor("rvh", [H, H * C], BF16, kind="ExternalInput").ap()
    # packed int8 y, one tensor per batch (n) so the host can decode y0
    # while y1 is still on the wire: per (c) row = 16384 int8 q values
    # (i-major) followed by 128 bf16 per-i scales (as raw bytes);
    # host: y = q * scale[c,i]
    y_aps = [nc.dram_tensor(f"y{n}", [C, H * (W + 2)], I8,
                            kind="ExternalOutput").ap()
             for n in range(NPC)]

    from contextlib import ExitStack
    with tile.TileContext(nc) as tc, ExitStack() as ctx:
        wpool = ctx.enter_context(tc.tile_pool(name="w", bufs=1))
        big = ctx.enter_context(tc.tile_pool(name="big", bufs=1))
        chunk = ctx.enter_context(tc.tile_pool(name="chunk", bufs=4))
        small = ctx.enter_context(tc.tile_pool(name="small", bufs=2))
        xstg = ctx.enter_context(tc.tile_pool(name="xstg", bufs=3))
        pp = ctx.enter_context(tc.tile_pool(name="pp", bufs=6, space="PSUM"))

        wq = wpool.tile([C, C], BF16, tag="wq")
        wk = wpool.tile([C, C], BF16, tag="wk")
        wv = wpool.tile([C, C], BF16, tag="wv")
        nc.sync.dma_start(wq[:], wq_ap[:])
        nc.sync.dma_start(wk[:], wk_ap[:])
        nc.sync.dma_start(wv[:], wv_ap[:])

        for n in range(NPC):
            # ---- stage A: load int8 x, dequant to bf16, project q/k, vT ----
            # decode the per-partition scale 2**((c0*127+c1)/8)
            sc8 = small.tile([C, 2], I8, tag="sc8")
            nc.sync.dma_start(sc8[:], x_ap[n][:, HW:HW + 2])
            scf = small.tile([C, 2], F32, tag="scf")
            nc.scalar.copy(scf[:], sc8[:])
            sexp = small.tile([C, 2], F32, tag="sexp")
            nc.vector.scalar_tensor_tensor(
                sexp[:, 0:1], scf[:, 0:1], 127.0, scf[:, 1:2],
                op0=mybir.AluOpType.mult, op1=mybir.AluOpType.add)
            nc.scalar.activation(sexp[:, 1:2], sexp[:, 0:1],
                                 mybir.ActivationFunctionType.Exp,
                                 scale=LN2_8)
            xb = big.tile([C, HW], BF16, tag="x_eq")     # also Eq's slot later
            for s in range(8):
                stg = xstg.tile([C, 2048], I8, tag="stg")
                nc.sync.dma_start(stg[:], x_ap[n][:, s * 2048:(s + 1) * 2048])
                nc.scalar.activation(xb[:, s * 2048:(s + 1) * 2048], stg[:],
                                     mybir.ActivationFunctionType.Copy,
                                     scale=sexp[:, 1:2])
            qb = big.tile([C, HW], BF16, tag="qb")
            kb = big.tile([C, HW], BF16, tag="kb")
            for s in range(HW // 512):
                ps = pp.tile([128, 512], F32, tag="ps")
                nc.tensor.matmul(ps[:], wq[:], xb[:, s * 512:(s + 1) * 512])
                nc.scalar.copy(qb[:, s * 512:(s + 1) * 512], ps[:])
                ps2 = pp.tile([128, 512], F32, tag="ps")
                nc.tensor.matmul(ps2[:], wk[:], xb[:, s * 512:(s + 1) * 512])
                nc.scalar.copy(kb[:, s * 512:(s + 1) * 512], ps2[:])
            vT = big.tile([H, W * C], BF16, tag="vT")    # [h,(j,c)]
            for j0 in range(0, W, 4):
                ps = pp.tile([128, 512], F32, tag="ps")
                for jj in range(4):
                    j = j0 + jj
                    nc.tensor.matmul(ps[:, jj * C:(jj + 1) * C],
                                     xb[:, j::W], wv[:])
                if (j0 // 4) % 2 == 0:
                    nc.vector.tensor_copy(vT[:, j0 * C:(j0 + 4) * C], ps[:])
                else:
                    nc.scalar.copy(vT[:, j0 * C:(j0 + 4) * C], ps[:])

            # ---- stage C: qk -> Eq = exp(qk), layout [h,(i,j)] -------------
            Eq = big.tile([H, HW], BF16, tag="x_eq")
            Eq_ji = Eq[:].rearrange("p (i j) -> p j i", j=W)
            for j0 in range(0, W, 4):
                ps = pp.tile([128, 512], F32, tag="ps")
                for jj in range(4):
                    j = j0 + jj
                    nc.tensor.matmul(ps[:, jj * H:(jj + 1) * H],
                                     kb[:, j::W], qb[:, j::W])
                nc.scalar.activation(Eq_ji[:, j0:j0 + 4, :], ps[:],
                                     mybir.ActivationFunctionType.Exp)

            # ---- stage B (fused): Sr -> E -> sigma -> 1/sigma -> Wn -> out2
            outb = big.tile([C, HW], BF16, tag="out")
            sig = small.tile([H, H], F32, tag="sig")
            rec = small.tile([H, H], F32, tag="rec")
            def emit_out2(i0, rvc):
                # out2 for a whole 32-i block (emitted one block late so PE
                # never waits on this block's just-finished normalize)
                for i1 in range(0, ICHUNK, 4):
                    i = i0 + i1
                    ps2 = pp.tile([128, 512], F32, tag="ps")
                    for ii in range(4):
                        il = i1 + ii
                        nc.tensor.matmul(ps2[:, ii * W:(ii + 1) * W],
                                         rvc[:, il * C:(il + 1) * C],
                                         Eq[:, (i + ii) * W:(i + ii + 1) * W])
                    nc.scalar.copy(outb[:, i * W:(i + 4) * W], ps2[:])

            prev = None
            for i0 in range(0, H, ICHUNK):
                rqc = chunk.tile([C, ICHUNK * H], BF16, tag="chunk")
                nc.sync.dma_start(rqc[:], rq_ap[:, i0 * H:(i0 + ICHUNK) * H])
                rkc = chunk.tile([C, ICHUNK * H], BF16, tag="chunk")
                nc.sync.dma_start(rkc[:], rk_ap[:, i0 * H:(i0 + ICHUNK) * H])
                rvc = chunk.tile([H, ICHUNK * C], BF16, tag="chunk")
                nc.sync.dma_start(rvc[:], rv_ap[:, i0 * C:(i0 + ICHUNK) * C])
                for i1 in range(0, ICHUNK, 4):
                    i = i0 + i1
                    ps = pp.tile([128, 512], F32, tag="ps")
                    for ii in range(4):
                        il = i1 + ii
                        nc.tensor.matmul(ps[:, ii * W:(ii + 1) * W],
                                         rqc[:, il * H:(il + 1) * H],
                                         qb[:, (i + ii) * W:(i + ii + 1) * W],
                                         start=True, stop=False)
                        nc.tensor.matmul(ps[:, ii * W:(ii + 1) * W],
                                         rkc[:, il * H:(il + 1) * H],
                                         kb[:, (i + ii) * W:(i + ii + 1) * W],
                                         start=False, stop=True)
                    st = small.tile([128, 512], BF16, tag="stemp")
                    nc.scalar.activation(st[:], ps[:],
                                         mybir.ActivationFunctionType.Exp)
                    # E = Eq*exp(Sr) fused with sigma accumulation, per i
                    for ii in range(4):
                        nc.vector.scalar_tensor_tensor(
                            Eq[:, (i + ii) * W:(i + ii + 1) * W],
                            Eq[:, (i + ii) * W:(i + ii + 1) * W],
                            1.0, st[:, ii * W:(ii + 1) * W],
                            op0=mybir.AluOpType.mult,
                            op1=mybir.AluOpType.mult,
                            accum_out=sig[:, i + ii:i + ii + 1])
                    nc.vector.reciprocal(rec[:, i:i + 4], sig[:, i:i + 4])
                    for ii in range(4):
                        nc.vector.tensor_scalar_mul(
                            Eq[:, (i + ii) * W:(i + ii + 1) * W],
                            Eq[:, (i + ii) * W:(i + ii + 1) * W],
                            rec[:, i + ii:i + ii + 1])
                if prev is not None:
                    emit_out2(*prev)
                prev = (i0, rvc)
            emit_out2(*prev)

            # ---- stage F: out1 (per-j, strided add) ------------------------
            Wn_ij = Eq[:].rearrange("p (i j) -> p i j", j=W)
            out_ji = outb[:].rearrange("p (i j) -> p j i", j=W)
            for j0 in range(0, W, 4):
                ps = pp.tile([128, 512], F32, tag="ps")
                for jj in range(4):
                    j = j0 + jj
                    nc.tensor.matmul(ps[:, jj * H:(jj + 1) * H],
                                     vT[:, j * C:(j + 1) * C],
                                     Wn_ij[:, :, j])
                nc.vector.tensor_add(
                    out_ji[:, j0:j0 + 4, :], out_ji[:, j0:j0 + 4, :],
                    ps[:].rearrange("p (a b) -> p a b", b=H))

            # ---- stage G: quantize outb -> int8 q + bf16 per-(c,i) scales --
            # SCALE_Q slightly under 127 keeps |q| < 127.5 under f32
            # reciprocal rounding, so the int8 convert can never wrap.
            mxo = small.tile([C, H], F32, tag="mxo")
            nc.vector.tensor_reduce(
                mxo[:], outb[:].rearrange("p (i j) -> p i j", j=W),
                axis=mybir.AxisListType.X, op=mybir.AluOpType.max,
                apply_absolute_value=True)
            nc.vector.tensor_scalar_max(mxo[:], mxo[:], 1e-30)
            recq = small.tile([C, H], F32, tag="recq")
            nc.vector.reciprocal(recq[:], mxo[:])
            ssc = small.tile([C, H], BF16, tag="ssc")
            nc.scalar.activation(ssc[:], mxo[:],
                                 mybir.ActivationFunctionType.Copy,
                                 scale=1.0 / SCALE_Q)
            q8 = big.tile([C, HW], I8, tag="vT")   # reuses vT's slot
            for i in range(H):
                nc.vector.tensor_scalar(
                    q8[:, i * W:(i + 1) * W], outb[:, i * W:(i + 1) * W],
                    recq[:, i:i + 1], SCALE_Q,
                    op0=mybir.AluOpType.mult, op1=mybir.AluOpType.mult)
            for s in range(4):
                nc.sync.dma_start(y_aps[n][:, s * 4096:(s + 1) * 4096],
                                  q8[:, s * 4096:(s + 1) * 4096])
            nc.sync.dma_start(y_aps[n][:, HW:HW + 2 * H].bitcast(BF16),
                              ssc[:])

    nc.compile()
    return nc


def _get_prog():
    global _PROG
    if _PROG is None:
        _PROG = _build()
    return _PROG


def _quant_pack_into(xn, buf_n):
    """Quantize one batch: f32 (C, HW) -> packed int8 (C, HW+2) written
    into buf_n, with power-of-two exponent scales
    (value = (c0*127+c1) eighths of an octave)."""
    mx = np.abs(xn).max(axis=1, keepdims=True)
    np.maximum(mx, 1e-30, out=mx)
    et = np.round(np.log2(mx / 127.0) * 8.0)
    c0 = np.clip(np.round(et / 127.0), -126, 126)
    c1 = et - c0 * 127.0
    s = np.exp2((c0 * 127.0 + c1) * 0.125).astype(np.float32)
    q = xn / s
    np.rint(q, out=q)
    np.clip(q, -127, 127, out=q)
    buf_n[:, :HW] = q                            # exact: rint'd floats
    buf_n[:, HW] = c0[:, 0]
    buf_n[:, HW + 1] = c1[:, 0]


def _quant_pack_x(xf):
    """f32 (B, C, HW) -> packed int8 (B, C, HW+2)."""
    buf = np.empty((xf.shape[0], C, HW + 2), np.int8)
    for n in range(xf.shape[0]):
        _quant_pack_into(xf[n], buf[n])
    return buf


def _prep_inputs(x, Wq, Wk, Wv, rq, rk, rv, Gq, Gk, Gv1, Gv2):
    bf = ml_dtypes.bfloat16
    d = np.float32(np.sqrt(C))
    wqt = np.ascontiguousarray((Wq / d).T).astype(bf)
    wkt = np.ascontiguousarray(Wk.T).astype(bf)
    wvt = np.ascontiguousarray((Gv1[0] * Wv).T).astype(bf)
    rqh = np.ascontiguousarray((Gq[0] * rq).transpose(0, 2, 1)).reshape(C, HW).astype(bf)
    rkh = np.ascontiguousarray((Gk[0] / d * rk).transpose(0, 2, 1)).reshape(C, HW).astype(bf)
    rvh = np.ascontiguousarray((Gv2[0] * rv).transpose(1, 2, 0)).reshape(H, H * C).astype(bf)
    xb = _quant_pack_x(np.ascontiguousarray(x, np.float32).reshape(N, C, HW))
    return xb, wqt, wkt, wvt, rqh, rkh, rvh


# ---------------------------------------------------------------------------
# Fast dispatch path: cached jits + int8 transport over the axon tunnel.
# ---------------------------------------------------------------------------

def _get_state():
    global _STATE
    if _STATE is not None:
        return _STATE
    with _LOCK:
        if _STATE is not None:
            return _STATE
        nc = _get_prog()
        bass2jax.install_neuronx_cc_hook()
        _bass_exec_p = bass2jax._bass_exec_p
        partition_id_tensor = bass2jax.partition_id_tensor

        partition_name = (nc.partition_id_tensor.name
                          if nc.partition_id_tensor else None)
        in_names, out_names, out_avals = [], [], []
        for alloc in nc.m.functions[0].allocations:
            if not isinstance(alloc, mybir.MemoryLocationSet):
                continue
            name = alloc.memorylocations[0].name
            if alloc.kind == "ExternalInput":
                if name != partition_name:
                    in_names.append(name)
            elif alloc.kind == "ExternalOutput":
                out_names.append(name)
                out_avals.append(jax.core.ShapedArray(
                    tuple(alloc.tensor_shape), mybir.dt.np(alloc.dtype)))
        assert in_names == ["x2", "wqt", "wkt", "wvt", "rqh", "rkh", "rvh"], in_names
        assert out_names == ["y0", "y1"], out_names
        n_params = len(in_names)
        all_in = in_names + out_names + (
            [partition_name] if partition_name else [])

        def _body(*args):
            ops = list(args)
            if partition_name is not None:
                ops.append(partition_id_tensor())
            return tuple(_bass_exec_p.bind(
                *ops, out_avals=tuple(out_avals), in_names=tuple(all_in),
                out_names=tuple(out_names), lowering_input_output_aliases=(),
                sim_require_finite=True, sim_require_nnan=True, nc=nc))

        devices = jax.devices()[:N_CORES]
        mesh = Mesh(np.asarray(devices), ("core",))
        shard8 = NamedSharding(mesh, P("core"))
        repl = NamedSharding(mesh, P())
        in_specs = (P("core"),) + (P(None),) * 6 + (P("core"), P("core"))
        try:
            from jax import shard_map as _shard_map
            smap = _shard_map(_body, mesh=mesh, in_specs=in_specs,
                              out_specs=(P("core"), P("core")),
                              check_vma=False)
        except Exception:
            from jax.experimental.shard_map import shard_map as _shard_map
            smap = _shard_map(_body, mesh=mesh, in_specs=in_specs,
                              out_specs=(P("core"), P("core")),
                              check_rep=False)
        exec_j = jax.jit(smap, donate_argnums=(n_params, n_params + 1),
                         keep_unused=True)

        # Both directions travel as int8 arrays with scales embedded in
        # the payload (x: power-of-two exponent columns, y: bf16 scale
        # bytes), quantized/dequantized inside the bass kernel itself —
        # steady state is a single exec dispatch per call. Called twice at
        # bootstrap so the two donated buffers are distinct.
        mk_zeros = jax.jit(
            lambda: jnp.zeros((N_CORES * C, H * (W + 2)), jnp.int8),
            out_shardings=shard8)

        _STATE = {
            "nc": nc, "devices": devices, "mesh": mesh, "shard8": shard8,
            "repl": repl, "exec_j": exec_j,
            "mk_zeros": mk_zeros, "donate": None,
            "wcache_key": None, "wcache_dev": None,
        }
    return _STATE


def _prep_weights(st, Wq, Wk, Wv, rq, rk, rv, Gq, Gk, Gv1, Gv2):
    """Device-resident replicated params, revalidated exactly per call."""
    key = (Wq, Wk, Wv, rq, rk, rv, Gq, Gk, Gv1, Gv2)
    ck = st["wcache_key"]
    if ck is not None and all(
            a.shape == b.shape and a.dtype == b.dtype and np.array_equal(a, b)
            for a, b in zip(ck, key)):
        return st["wcache_dev"]
    bf = ml_dtypes.bfloat16
    d = np.float32(np.sqrt(C))
    wqt = np.ascontiguousarray((Wq / d).T).astype(bf)
    wkt = np.ascontiguousarray(Wk.T).astype(bf)
    wvt = np.ascontiguousarray((Gv1[0] * Wv).T).astype(bf)
    rqh = np.ascontiguousarray((Gq[0] * rq).transpose(0, 2, 1)
                               ).reshape(C, HW).astype(bf)
    rkh = np.ascontiguousarray((Gk[0] / d * rk).transpose(0, 2, 1)
                               ).reshape(C, HW).astype(bf)
    rvh = np.ascontiguousarray((Gv2[0] * rv).transpose(1, 2, 0)
                               ).reshape(H, H * C).astype(bf)
    d0 = st["devices"][0]
    # single tunnel transfer to core 0, then fast on-device broadcast
    dev = tuple(jax.device_put(jax.device_put(a, d0), st["repl"])
                for a in (wqt, wkt, wvt, rqh, rkh, rvh))
    jax.block_until_ready(dev)
    st["wcache_key"] = tuple(np.array(a, copy=True) for a in key)
    st["wcache_dev"] = dev
    return dev


def _quant_upload_x(st, xf):
    """Quantize+pack each batch in its own task (so the first core's
    buffer is ready ~2x sooner) and upload per-core chunks from a
    separate pool; host quantization overlaps the wire."""
    devices = st["devices"]
    bufs = [np.empty((NPC, C, HW + 2), np.int8) for _ in range(N_CORES)]
    with ThreadPoolExecutor(max_workers=8) as qex, \
            ThreadPoolExecutor(max_workers=4) as pex:
        qfuts = {}
        for c in range(N_CORES):
            for n in range(NPC):
                qfuts[(c, n)] = qex.submit(
                    _quant_pack_into, xf[c * NPC + n], bufs[c][n])

        def put(c):
            for n in range(NPC):
                qfuts[(c, n)].result()
            return jax.device_put(bufs[c], devices[c])
        shards = [f.result() for f in
                  [pex.submit(put, c) for c in range(N_CORES)]]
    return jax.make_array_from_single_device_arrays(
        (N, C, HW + 2), st["shard8"], shards)


def _decode_packed_y(ph, out):
    """ph (B, C, H*(W+2)) int8 rows = q bytes | bf16 scale bytes;
    out (B, C, H, W) f32 view to write into."""
    b = ph.shape[0]
    q = ph[:, :, :HW].reshape(b, C, H, W)
    s = np.ascontiguousarray(ph[:, :, HW:]).view(ml_dtypes.bfloat16)
    s = s.astype(np.float32).reshape(b, C, H, 1)
    np.multiply(q, s, out=out)


def _fetch_dequant_y(pk0, pk1):
    """Download the two packed int8 y halves (async-prefetched so the
    transfers run back to back); decode y0 with threads while y1 is
    still on the wire. Core c's shard of y<n> is global batch c*NPC+n."""
    for pk in (pk0, pk1):
        try:
            pk.copy_to_host_async()
        except Exception:
            pass
    out = np.empty((N, C, H, W), np.float32)

    def dec(ph, n, core):
        _decode_packed_y(ph[core:core + 1],
                         out[core * NPC + n:core * NPC + n + 1])
    with ThreadPoolExecutor(max_workers=8) as ex:
        f1 = ex.submit(lambda: np.asarray(pk1))  # materialize in parallel
        ph0 = np.asarray(pk0).reshape(N_CORES, C, H * (W + 2))
        futs = [ex.submit(dec, ph0, 0, c) for c in range(N_CORES)]
        ph1 = f1.result().reshape(N_CORES, C, H * (W + 2))
        futs += [ex.submit(dec, ph1, 1, c) for c in range(N_CORES)]
        for f in futs:
            f.result()
    return out


def _kernel_impl(x, Wq, Wk, Wv, rq, rk, rv, Gq, Gk, Gv1, Gv2):
    st = _get_state()
    arrs = [np.asarray(a, np.float32) for a in
            (Wq, Wk, Wv, rq, rk, rv, Gq, Gk, Gv1, Gv2)]
    wdev = _prep_weights(st, *arrs)
    xf = np.asarray(x, np.float32).reshape(N, C, HW)
    xbuf = _quant_upload_x(st, xf)
    donate = st["donate"]
    if donate is None:
        donate = (st["mk_zeros"](), st["mk_zeros"]())
    st["donate"] = None
    pk0, pk1 = st["exec_j"](xbuf, *wdev, *donate)
    out = _fetch_dequant_y(pk0, pk1)
    # the outputs' storage is recycled as the next call's donated output
    # buffers (the bass kernel overwrites every element; fetch completed)
    st["donate"] = (pk0, pk1)
    return out


def _reset_state():
    """Drop all cached device state/jits and re-create the PJRT client —
    recovery path for transient runtime faults (e.g. NRT exec-unit
    errors that leave the in-process client unusable)."""
    global _STATE
    _STATE = None
    try:
        jax.clear_caches()
    except Exception:
        pass
    try:
        import jax._src.xla_bridge as _xb
        _xb._clear_backends()
    except Exception:
        pass


def kernel(x, Wq, Wk, Wv, rq, rk, rv, Gq, Gk, Gv1, Gv2):
    try:
        return _kernel_impl(x, Wq, Wk, Wv, rq, rk, rv, Gq, Gk, Gv1, Gv2)
    except Exception:
        _reset_state()
        return _kernel_impl(x, Wq, Wk, Wv, rq, rk, rv, Gq, Gk, Gv1, Gv2)
